# revision 2
# baseline (speedup 1.0000x reference)
"""KroneckerLinear Trainium2 kernel (bf16, transpose-free dataflow, v2).

y[b,t,o*64+q] = sum_{s,i,j} A[s,o,i] * x[b,t,i*64+j] * B[s,q,j] + bias[o*64+q]

Data-parallel over the 16384 tokens, 2048 per core. Per token t the op is
Y_t = sum_s A_s @ X_t @ B_s^T with X_t = x_t.reshape(64,64).

On-chip dataflow per 16-token tile, all matmuls bf16 in the UNIFORM 64x64 PE
tiling mode (mixing tiling modes forces a full-array drain per switch):
  MM1 (16x): per (c, rho, tau) quadrant: ST[tau-half(j), rho*512+c*128+(s,o)]
             = sum_i X_token[i, j] * A2[i, (s,o)]; stationary = the token's
             X (64x64), moving = A2 (fixed). rho picks the PSUM bank, tau the
             partitions, so concurrent matmuls never share (partition, bank).
  MM2 (4x):  ST[tau-half(q), 1024 + (r,c,o)] += over s: B_s^T[j,q] @ G-slice
             of tile n-2 (read back from SBUF in bf16). The Kronecker "swap"
             is free: MM2's moving operand is a strided 3-dim AP.
  evac:      ONE 3-bank PSUM supertile [U(banks 0-1) | Y(bank 2)] per tile is
             drained by exactly two ops: ScalarE copies cols [0:SPLIT],
             VectorE cols [SPLIT:1536], f32 -> bf16, into a 16-slot
             persistent SBUF ring (slot = [U 1024 | Y 512] bf16). SPLIT is
             chosen so both engines take ~equal time ((222+FD)/1.2 vs
             (120+FD)/0.96 ns). The slot's U half feeds MM2 two tiles later;
             the Y half (data of tile n-2) is DMA'd straight to HBM with a
             strided [128, slots, 512] AP -- no separate Y staging copy.

The PSUM evacuation through the only two PSUM-capable engines is the
structural bottleneck (12288 f32 PSUM reads per token / 128 lanes), so the
whole kernel is arranged to keep those two engines dense: one instruction
each per tile (amortizing the ~120-220 cycle read-write bubble), balanced
column split, and all other traffic (input DMA on the SP HWDGE ring, output
DMA on the GpSimd SWDGE ring) off their queues.

Host does the (free, unmeasured) layout shuffles, f32<->bf16 conversion and
the bias add.
"""

import numpy as np
import ml_dtypes

IN1 = IN2 = OUT1 = OUT2 = 64
NUM_SUM = 2
BATCH, SEQ = 4, 4096
NCORES = 8
TOK = BATCH * SEQ            # 16384 tokens
TPC = TOK // NCORES          # 2048 tokens per core
TILE_TOK = 16                # tokens per on-chip tile
NT = TPC // TILE_TOK         # 128 tiles per core
GRP = 8                      # tiles per input DMA group / output DMA group
NSLOT = 16                   # SBUF ring slots (2 output groups in flight)
SPLIT = 836                  # ScalarE/VectorE evacuation column split
MM2_LAG = 2                  # tiles between MM1 and the MM2 consuming it

BF16 = ml_dtypes.bfloat16

_cached = {}


def _build_bass(nt=NT):
    import concourse.bass as bass  # noqa: F401
    import concourse.mybir as mybir
    from concourse import bacc, tile

    f32 = mybir.dt.float32
    bf16 = mybir.dt.bfloat16
    nc = bacc.Bacc(None, target_bir_lowering=False, debug=False)

    assert nt % NSLOT == 0 and nt >= 3 * GRP
    xdev = nc.declare_dram_parameter("xdev", [128, nt * 512], bf16, isOutput=False)
    a2d = nc.declare_dram_parameter("a2d", [128, 128], bf16, isOutput=False)
    b2d = nc.declare_dram_parameter("b2d", [128, 128], bf16, isOutput=False)
    ydev = nc.declare_dram_parameter("ydev", [128, nt * 512], bf16, isOutput=True)

    with tile.TileContext(nc) as tc:
        with (
            tc.tile_pool(name="consts", bufs=1) as cpool,
            tc.tile_pool(name="xs", bufs=4) as xpool,
            tc.tile_pool(name="sts", bufs=2, space="PSUM") as stpool,
        ):
            a2 = cpool.tile([128, 128], bf16)
            b2 = cpool.tile([128, 128], bf16)
            nc.sync.dma_start(out=a2, in_=a2d[:, :])
            nc.sync.dma_start(out=b2, in_=b2d[:, :])

            # 16-slot SBUF ring; slot n%16 = [U(n) bf16 1024 | Y(n-2) bf16 512].
            gy = cpool.tile([128, NSLOT * 1536], bf16)

            def gy_slot(k):
                return gy[:, k * 1536:(k + 1) * 1536]

            def gy_y_span(k0, k1):
                # [128, k1-k0, 512] strided AP over slots' Y halves
                g3 = gy[:, :].rearrange("a (k c) -> a k c", k=NSLOT, c=1536)
                return g3[:, k0:k1, 1024:1536]

            def emit_mm2(n, st):
                # MM2 for data tile n-MM2_LAG, into st's Y bank (cols 1024+).
                slot2 = (n - MM2_LAG) % NSLOT
                g5 = gy_slot(slot2)[:, 0:1024].rearrange(
                    "a (r c s o) -> a s r c o", r=2, c=4, s=2, o=64)
                for tau in range(2):
                    for s in range(2):
                        nc.tensor.matmul(
                            st[tau * 64:(tau + 1) * 64, 1024:1536],
                            lhsT=b2[tau * 64:(tau + 1) * 64,
                                    s * 64:(s + 1) * 64],
                            rhs=g5[tau * 64:(tau + 1) * 64, s],
                            start=(s == 0), stop=(s == 1),
                            tile_position=(tau * 64, tau * 64),
                        )

            def emit_out_dma(m):
                # Ships Y of tiles [8m-2 .. 8m+5] (ring slots 8m..8m+7, lagged
                # by MM2_LAG) except the first group, which has no Y for the
                # two warmup slots.
                s0 = (GRP * m) % NSLOT
                if m == 0:
                    k0, k1 = MM2_LAG, GRP
                    ybase = 0
                else:
                    k0, k1 = s0, s0 + GRP
                    ybase = GRP * m - MM2_LAG
                ylen = k1 - k0
                nc.gpsimd.dma_start(
                    out=ydev[:, ybase * 512:(ybase + ylen) * 512],
                    in_=gy_y_span(k0, k1))

            # Variable input group sizes: small groups at the head so compute
            # starts after a ~128 KB DMA instead of 1 MB.
            head = [1, 1, 2, 4]
            tail = [4, 2, 1, 1]
            mid = (nt - sum(head) - sum(tail)) // GRP
            sizes = head + [GRP] * mid + tail
            assert sum(sizes) == nt, (sizes, nt)

            base = 0
            for glen in sizes:
                xs = xpool.tile([128, GRP * 512], bf16, tag="xs")
                nc.sync.dma_start(
                    out=xs[:, 0:glen * 512],
                    in_=xdev[:, base * 512:(base + glen) * 512])

                for t in range(glen):
                    n = base + t
                    st = stpool.tile([128, 1536], f32, tag="st")

                    # MM1: 16 matmuls, uniform 64x64 PE tiling. Quadrant
                    # (rho, tau) holds token 16g+4c+2rho+tau's X stationary;
                    # rho picks the PSUM bank, tau the output partitions.
                    for c in range(4):
                        for rho in range(2):
                            for tau in range(2):
                                nc.tensor.matmul(
                                    st[tau * 64:(tau + 1) * 64,
                                       rho * 512 + c * 128:
                                       rho * 512 + (c + 1) * 128],
                                    lhsT=xs[rho * 64:(rho + 1) * 64,
                                            t * 512 + c * 128 + tau * 64:
                                            t * 512 + c * 128 + (tau + 1) * 64],
                                    rhs=a2[rho * 64:(rho + 1) * 64, :],
                                    start=True, stop=True,
                                    tile_position=(rho * 64, tau * 64),
                                )

                    if n >= MM2_LAG:
                        emit_mm2(n, st)

                    # Supertile drain: one op per engine, split for equal
                    # engine time. For warmup tiles the Y bank is untouched,
                    # so only the U columns are copied.
                    dst = gy_slot(n % NSLOT)
                    hi = 1536 if n >= MM2_LAG else 1024
                    nc.scalar.copy(dst[:, 0:SPLIT], st[:, 0:SPLIT])
                    nc.vector.tensor_copy(dst[:, SPLIT:hi], st[:, SPLIT:hi])

                    if n % GRP == GRP - 1:
                        emit_out_dma(n // GRP)
                base += glen

            # Epilogue: MM2 + Y drain for the last MM2_LAG tiles.
            for k in range(MM2_LAG):
                e = nt + k
                st = stpool.tile([128, 1536], f32, tag="st")
                emit_mm2(e, st)
                dst = gy_slot(e % NSLOT)
                if k % 2 == 0:
                    nc.scalar.copy(dst[:, 1024:1536], st[:, 1024:1536])
                else:
                    nc.vector.tensor_copy(dst[:, 1024:1536], st[:, 1024:1536])
            nc.gpsimd.dma_start(
                out=ydev[:, (nt - MM2_LAG) * 512:nt * 512],
                in_=gy_y_span(0, MM2_LAG))

    nc.finalize()
    return nc


def _get_nc(nt=NT):
    key = ("nc", nt)
    if key not in _cached:
        _cached[key] = _build_bass(nt)
    return _cached[key]


def _host_prep_x(xc):
    # xc: (TPC, 4096) f32 ->
    # xdev[rho*64+i, g*512 + c*128 + tau*64 + j] = xc[16g + 4c + 2rho + tau, i*64+j]
    x6 = xc.astype(BF16).reshape(NT, 4, 2, 2, IN1, IN2)   # g, c, rho, tau, i, j
    xd = x6.transpose(2, 4, 0, 1, 3, 5)                   # rho, i, g, c, tau, j
    return np.ascontiguousarray(xd).reshape(128, NT * 512)


def _host_post_y(yd, bias):
    # yd: (128, NT*512) bf16;
    # ydev[tau*64+q, g*512 + r*256 + c*64 + o] = y_mm[16g + 4c + 2r + tau, o*64+q]
    # bias is added here in f32 as part of the unpack epilogue.
    y6 = yd.reshape(2, OUT2, NT, 2, 4, OUT1)              # tau, q, g, r, c, o
    yc = y6.transpose(2, 4, 3, 0, 5, 1)                   # g, c, r, tau, o, q
    out = np.ascontiguousarray(yc).reshape(TPC, OUT1 * OUT2).astype(np.float32)
    out += bias
    return out


def _make_in_maps(x, A, B, bias):
    A = np.asarray(A, np.float32)
    B = np.asarray(B, np.float32)
    bias = np.asarray(bias, np.float32)
    xf = np.ascontiguousarray(x, np.float32).reshape(TOK, IN1 * IN2)

    at = A.transpose(2, 0, 1).reshape(IN1, NUM_SUM * OUT1)     # i, (s,o)
    a2d = np.ascontiguousarray(np.concatenate([at, at], 0)).astype(BF16)
    bt = B.transpose(2, 0, 1).reshape(IN2, NUM_SUM * OUT2)     # j, (s,q)
    b2d = np.ascontiguousarray(np.concatenate([bt, bt], 0)).astype(BF16)

    in_maps = []
    for cid in range(NCORES):
        xc = xf[cid * TPC:(cid + 1) * TPC]
        in_maps.append({
            "xdev": _host_prep_x(xc),
            "a2d": a2d,
            "b2d": b2d,
        })
    return in_maps


def _run(inputs, trace=False, **kw):
    from concourse.bass_utils import run_bass_kernel_spmd

    nc = _get_nc()
    in_maps = _make_in_maps(**inputs)
    res = run_bass_kernel_spmd(nc, in_maps, core_ids=list(range(NCORES)),
                               trace=trace, **kw)
    bias_f32 = np.asarray(inputs["bias"], np.float32)
    shards = [_host_post_y(np.asarray(res.results[c]["ydev"]), bias_f32)
              for c in range(NCORES)]
    y = np.concatenate(shards, 0).reshape(BATCH, SEQ, OUT1 * OUT2)
    return y, res


def kernel(x, A, B, bias):
    y, _ = _run(dict(x=x, A=A, B=B, bias=bias), trace=False)
    return y


# revision 3
# speedup vs baseline: 1.3978x; 1.3978x over previous
"""KroneckerLinear Trainium2 kernel (bf16, transpose-free dataflow, v3).

y[b,t,o*64+q] = sum_{s,i,j} A[s,o,i] * x[b,t,i*64+j] * B[s,p,j] + bias[o*64+q]

Data-parallel over the 16384 tokens, 2048 per core. Per token t the op is
Y_t = sum_s A_s @ X_t @ B_s^T with X_t = x_t.reshape(64,64).

On-chip dataflow per 16-token tile, all matmuls bf16 in the UNIFORM 64x64 PE
tiling mode (mixing tiling modes forces a full-array drain per switch):
  MM1 (16x): per (c, rho, tau) quadrant: U[tau-half(j), rho*512+c*128+(s,o)]
             = sum_i X_token[i, j] * A2[i, (s,o)]; stationary = the token's
             X (64x64), moving = A2 (fixed). rho picks the PSUM bank, tau the
             partitions, so concurrent matmuls never share (partition, bank).
  MM2 (4x):  Ypair[tau-half(q), (n%2)*512 + (r,c,o)] += over s:
             B_s^T[j,q] @ G-slice of tile n-2 (read back from SBUF in bf16).
             The Kronecker "swap" is free: MM2's moving operand is a strided
             3-dim AP. Two consecutive tiles' Y land in one 2-bank PSUM pair.
  evac:      3 ops per 2 tiles, every op 1024 f32 columns: U(2m), U(2m+1)
             (each its own 2-bank PSUM tile), and Ypair(2m) (2-bank tile,
             written to the two gy slots' Y ranges with a strided dest AP).
             Ops rotate over ScalarE/DVE 8:7 so both engines run ~equal time
             ((222+FD)/1.2 vs (120+FD)/0.96 ns); an op never shares a PSUM
             tile or bank with a concurrent op on the other engine (the tile
             scheduler serializes same-bank PSUM access between engines).

The PSUM evacuation through the only two PSUM-capable engines is the
structural bottleneck (12288 f32 PSUM reads per token / 128 lanes), so the
whole kernel is arranged to keep those two engines dense: uniformly big
1024-col ops (amortize the ~120-222 cycle read-write bubble), balanced
rotation, and all other traffic (input DMA on the SP HWDGE ring, output DMA
on the GpSimd SWDGE ring) off their queues. The bf16 evacuations land in a
16-slot persistent SBUF ring (slot = [U 1024 | Y 512]); the U half feeds MM2
two tiles later, and the Y halves are DMA'd straight to HBM with a strided
[128, slots, 512] AP, one SWDGE descriptor batch per 8 tiles.

Host does the (free, unmeasured) layout shuffles, f32<->bf16 conversion and
the bias add.
"""

import numpy as np
import ml_dtypes

IN1 = IN2 = OUT1 = OUT2 = 64
NUM_SUM = 2
BATCH, SEQ = 4, 4096
NCORES = 8
TOK = BATCH * SEQ            # 16384 tokens
TPC = TOK // NCORES          # 2048 tokens per core
TILE_TOK = 16                # tokens per on-chip tile
NT = TPC // TILE_TOK         # 128 tiles per core
GRP = 8                      # tiles per input DMA group / output DMA group
NSLOT = 16                   # SBUF ring slots (2 output groups in flight)
MM2_LAG = 2                  # tiles between MM1 and the MM2 consuming it

BF16 = ml_dtypes.bfloat16

_cached = {}


def _build_bass(nt=NT):
    import concourse.bass as bass  # noqa: F401
    import concourse.mybir as mybir
    from concourse import bacc, tile

    f32 = mybir.dt.float32
    bf16 = mybir.dt.bfloat16
    nc = bacc.Bacc(None, target_bir_lowering=False, debug=False)

    assert nt % NSLOT == 0 and nt >= 3 * GRP
    xdev = nc.declare_dram_parameter("xdev", [128, nt * 512], bf16, isOutput=False)
    a2d = nc.declare_dram_parameter("a2d", [128, 128], bf16, isOutput=False)
    b2d = nc.declare_dram_parameter("b2d", [128, 128], bf16, isOutput=False)
    ydev = nc.declare_dram_parameter("ydev", [128, nt * 512], bf16, isOutput=True)

    with tile.TileContext(nc) as tc:
        with (
            tc.tile_pool(name="consts", bufs=1) as cpool,
            tc.tile_pool(name="xs", bufs=4) as xpool,
            tc.tile_pool(name="ups", bufs=2, space="PSUM") as upool,
            tc.tile_pool(name="yps", bufs=2, space="PSUM") as ypool,
        ):
            a2 = cpool.tile([128, 128], bf16)
            b2 = cpool.tile([128, 128], bf16)
            nc.sync.dma_start(out=a2, in_=a2d[:, :])
            nc.sync.dma_start(out=b2, in_=b2d[:, :])

            # 16-slot SBUF ring; slot n%16 = [U(n) bf16 1024 | Y(n-2) bf16 512].
            gy = cpool.tile([128, NSLOT * 1536], bf16)
            g3 = gy[:, :].rearrange("a (k c) -> a k c", k=NSLOT, c=1536)

            # Rotating evacuation-engine assignment: 8 ScalarE ops per 7 DVE
            # ops balances (222+1024)/1.2 against (120+1024)/0.96.
            evac_k = [0]

            def evac(dst, src):
                k = evac_k[0] % 15
                evac_k[0] += 1
                if k % 2 == 0:
                    nc.scalar.copy(dst, src)
                else:
                    nc.vector.tensor_copy(dst, src)

            def emit_mm2(n, yp):
                # MM2 for data tile n-MM2_LAG, into Ypair half n%2.
                slot2 = (n - MM2_LAG) % NSLOT
                g5 = g3[:, slot2, 0:1024].rearrange(
                    "a (r c s o) -> a s r c o", r=2, c=4, s=2, o=64)
                half = (n % 2) * 512
                for tau in range(2):
                    for s in range(2):
                        nc.tensor.matmul(
                            yp[tau * 64:(tau + 1) * 64, half:half + 512],
                            lhsT=b2[tau * 64:(tau + 1) * 64,
                                    s * 64:(s + 1) * 64],
                            rhs=g5[tau * 64:(tau + 1) * 64, s],
                            start=(s == 0), stop=(s == 1),
                            tile_position=(tau * 64, tau * 64),
                        )

            def evac_ypair(n, yp):
                # Y data of tiles (n-1-MM2_LAG, n-MM2_LAG) -> Y ranges of gy
                # slots (n-1, n), one strided-dest op.
                k0 = (n - 1) % NSLOT
                dst = g3[:, k0:k0 + 2, 1024:1536]
                evac(dst, yp[:, :])

            def emit_out_dma(m):
                # Ships Y of tiles [8m-2 .. 8m+5] (ring slots 8m..8m+7, lagged
                # by MM2_LAG) except the first group, which has no Y for the
                # two warmup slots.
                s0 = (GRP * m) % NSLOT
                if m == 0:
                    k0, k1 = MM2_LAG, GRP
                    ybase = 0
                else:
                    k0, k1 = s0, s0 + GRP
                    ybase = GRP * m - MM2_LAG
                nc.gpsimd.dma_start(
                    out=ydev[:, ybase * 512:(ybase + (k1 - k0)) * 512],
                    in_=g3[:, k0:k1, 1024:1536])

            # Variable input group sizes: small groups at the head so compute
            # starts after a ~128 KB DMA instead of 1 MB.
            head = [1, 1, 2, 4]
            tail = [4, 2, 1, 1]
            mid = (nt - sum(head) - sum(tail)) // GRP
            sizes = head + [GRP] * mid + tail
            assert sum(sizes) == nt, (sizes, nt)

            ypair = None
            base = 0
            for glen in sizes:
                xs = xpool.tile([128, GRP * 512], bf16, tag="xs")
                nc.sync.dma_start(
                    out=xs[:, 0:glen * 512],
                    in_=xdev[:, base * 512:(base + glen) * 512])

                for t in range(glen):
                    n = base + t
                    ut = upool.tile([128, 1024], f32, tag="u")

                    # MM1: 16 matmuls, uniform 64x64 PE tiling. Quadrant
                    # (rho, tau) holds token 16g+4c+2rho+tau's X stationary;
                    # rho picks the PSUM bank, tau the output partitions.
                    for c in range(4):
                        for rho in range(2):
                            for tau in range(2):
                                nc.tensor.matmul(
                                    ut[tau * 64:(tau + 1) * 64,
                                       rho * 512 + c * 128:
                                       rho * 512 + (c + 1) * 128],
                                    lhsT=xs[rho * 64:(rho + 1) * 64,
                                            t * 512 + c * 128 + tau * 64:
                                            t * 512 + c * 128 + (tau + 1) * 64],
                                    rhs=a2[rho * 64:(rho + 1) * 64, :],
                                    start=True, stop=True,
                                    tile_position=(rho * 64, tau * 64),
                                )

                    if n >= MM2_LAG:
                        if n % 2 == 0:
                            ypair = ypool.tile([128, 1024], f32, tag="y")
                        emit_mm2(n, ypair)

                    # U evacuation: one 1024-col op into this tile's gy slot.
                    evac(g3[:, n % NSLOT, 0:1024], ut[:, :])
                    # Ypair evacuation once both halves are in.
                    if n >= MM2_LAG and n % 2 == 1:
                        evac_ypair(n, ypair)

                    if n % GRP == GRP - 1:
                        emit_out_dma(n // GRP)
                base += glen

            # Epilogue: MM2 + Y drain for the last MM2_LAG tiles.
            for k in range(MM2_LAG):
                e = nt + k
                if e % 2 == 0:
                    ypair = ypool.tile([128, 1024], f32, tag="y")
                emit_mm2(e, ypair)
                if e % 2 == 1:
                    evac_ypair(e, ypair)
            assert MM2_LAG == 2 and nt % 2 == 0
            nc.gpsimd.dma_start(
                out=ydev[:, (nt - MM2_LAG) * 512:nt * 512],
                in_=g3[:, 0:MM2_LAG, 1024:1536])

    nc.finalize()
    return nc


def _get_nc(nt=NT):
    key = ("nc", nt)
    if key not in _cached:
        _cached[key] = _build_bass(nt)
    return _cached[key]


def _host_prep_x(xc):
    # xc: (TPC, 4096) f32 ->
    # xdev[rho*64+i, g*512 + c*128 + tau*64 + j] = xc[16g + 4c + 2rho + tau, i*64+j]
    x6 = xc.astype(BF16).reshape(NT, 4, 2, 2, IN1, IN2)   # g, c, rho, tau, i, j
    xd = x6.transpose(2, 4, 0, 1, 3, 5)                   # rho, i, g, c, tau, j
    return np.ascontiguousarray(xd).reshape(128, NT * 512)


def _host_post_y(yd, bias):
    # yd: (128, NT*512) bf16;
    # ydev[tau*64+q, g*512 + r*256 + c*64 + o] = y_mm[16g + 4c + 2r + tau, o*64+q]
    # bias is added here in f32 as part of the unpack epilogue.
    y6 = yd.reshape(2, OUT2, NT, 2, 4, OUT1)              # tau, q, g, r, c, o
    yc = y6.transpose(2, 4, 3, 0, 5, 1)                   # g, c, r, tau, o, q
    out = np.ascontiguousarray(yc).reshape(TPC, OUT1 * OUT2).astype(np.float32)
    out += bias
    return out


def _make_in_maps(x, A, B, bias):
    A = np.asarray(A, np.float32)
    B = np.asarray(B, np.float32)
    bias = np.asarray(bias, np.float32)
    xf = np.ascontiguousarray(x, np.float32).reshape(TOK, IN1 * IN2)

    at = A.transpose(2, 0, 1).reshape(IN1, NUM_SUM * OUT1)     # i, (s,o)
    a2d = np.ascontiguousarray(np.concatenate([at, at], 0)).astype(BF16)
    bt = B.transpose(2, 0, 1).reshape(IN2, NUM_SUM * OUT2)     # j, (s,q)
    b2d = np.ascontiguousarray(np.concatenate([bt, bt], 0)).astype(BF16)

    in_maps = []
    for cid in range(NCORES):
        xc = xf[cid * TPC:(cid + 1) * TPC]
        in_maps.append({
            "xdev": _host_prep_x(xc),
            "a2d": a2d,
            "b2d": b2d,
        })
    return in_maps


def _run(inputs, trace=False, **kw):
    from concourse.bass_utils import run_bass_kernel_spmd

    nc = _get_nc()
    in_maps = _make_in_maps(**inputs)
    res = run_bass_kernel_spmd(nc, in_maps, core_ids=list(range(NCORES)),
                               trace=trace, **kw)
    bias_f32 = np.asarray(inputs["bias"], np.float32)
    shards = [_host_post_y(np.asarray(res.results[c]["ydev"]), bias_f32)
              for c in range(NCORES)]
    y = np.concatenate(shards, 0).reshape(BATCH, SEQ, OUT1 * OUT2)
    return y, res


def kernel(x, A, B, bias):
    y, _ = _run(dict(x=x, A=A, B=B, bias=bias), trace=False)
    return y


# revision 9
# speedup vs baseline: 1.4128x; 1.0107x over previous
"""KroneckerLinear Trainium2 kernel (bf16, transpose-free dataflow, v3).

y[b,t,o*64+q] = sum_{s,i,j} A[s,o,i] * x[b,t,i*64+j] * B[s,p,j] + bias[o*64+q]

Data-parallel over the 16384 tokens, 2048 per core. Per token t the op is
Y_t = sum_s A_s @ X_t @ B_s^T with X_t = x_t.reshape(64,64).

On-chip dataflow per 16-token tile, all matmuls bf16 in the UNIFORM 64x64 PE
tiling mode (mixing tiling modes forces a full-array drain per switch):
  MM1 (16x): per (c, rho, tau) quadrant: U[tau-half(j), rho*512+c*128+(s,o)]
             = sum_i X_token[i, j] * A2[i, (s,o)]; stationary = the token's
             X (64x64), moving = A2 (fixed). rho picks the PSUM bank, tau the
             partitions, so concurrent matmuls never share (partition, bank).
  MM2 (4x):  Y[tau-half(q), (r,c,o)] += over s: B_s^T[j,q] @ G-slice of tile
             n-2 (read back from SBUF in bf16). The Kronecker "swap" is
             free: MM2's moving operand is a strided 3-dim AP.
  evac:      2 ops per tile: the 1024-col U copy (ScalarE, except every 6th
             tile DVE) and the 512-col Y copy (DVE), sized so both engines
             run ~equal time ((222+FD)/1.2 vs (120+FD)/0.96 ns). An op never
             shares a PSUM tile or bank with a concurrent op on the other
             engine (the tile scheduler serializes same-bank PSUM access
             between engines). U tiles are triple-buffered (6 banks) and Y
             double-buffered (2 banks) so the PE can run 3 tiles ahead of
             the evacuation without stalling.

The PSUM evacuation through the only two PSUM-capable engines is the
structural bottleneck (12288 f32 PSUM reads per token / 128 lanes), so the
whole kernel is arranged to keep those two engines dense: uniformly big
1024-col ops (amortize the ~120-222 cycle read-write bubble), balanced
rotation, and all other traffic (input DMA on the SP HWDGE ring, output DMA
on the GpSimd SWDGE ring) off their queues. The bf16 evacuations land in a
16-slot persistent SBUF ring (slot = [U 1024 | Y 512]); the U half feeds MM2
two tiles later, and the Y halves are DMA'd straight to HBM with a strided
[128, slots, 512] AP, one SWDGE descriptor batch per 8 tiles.

Host does the (free, unmeasured) layout shuffles, f32<->bf16 conversion and
the bias add.
"""

import numpy as np
import ml_dtypes

IN1 = IN2 = OUT1 = OUT2 = 64
NUM_SUM = 2
BATCH, SEQ = 4, 4096
NCORES = 8
TOK = BATCH * SEQ            # 16384 tokens
TPC = TOK // NCORES          # 2048 tokens per core
TILE_TOK = 16                # tokens per on-chip tile
NT = TPC // TILE_TOK         # 128 tiles per core
GRP = 8                      # tiles per input DMA group / output DMA group
NSLOT = 16                   # SBUF ring slots (2 output groups in flight)
MM2_LAG = 2                  # tiles between MM1 and the MM2 consuming it

BF16 = ml_dtypes.bfloat16

_cached = {}


def _build_bass(nt=NT):
    import concourse.bass as bass  # noqa: F401
    import concourse.mybir as mybir
    from concourse import bacc, tile

    f32 = mybir.dt.float32
    bf16 = mybir.dt.bfloat16
    nc = bacc.Bacc(None, target_bir_lowering=False, debug=False)

    assert nt % NSLOT == 0 and nt >= 3 * GRP
    xdev = nc.declare_dram_parameter("xdev", [128, nt * 512], bf16, isOutput=False)
    a2d = nc.declare_dram_parameter("a2d", [128, 128], bf16, isOutput=False)
    b2d = nc.declare_dram_parameter("b2d", [128, 128], bf16, isOutput=False)
    ydev = nc.declare_dram_parameter("ydev", [128, nt * 512], bf16, isOutput=True)

    with tile.TileContext(nc) as tc:
        with (
            tc.tile_pool(name="consts", bufs=1) as cpool,
            tc.tile_pool(name="xs", bufs=4) as xpool,
            tc.tile_pool(name="ups", bufs=3, space="PSUM") as upool,
            tc.tile_pool(name="yps", bufs=2, space="PSUM") as ypool,
        ):
            a2 = cpool.tile([128, 128], bf16)
            b2 = cpool.tile([128, 128], bf16)
            nc.sync.dma_start(out=a2, in_=a2d[:, :])
            nc.sync.dma_start(out=b2, in_=b2d[:, :])

            # 16-slot SBUF ring; slot n%16 = [U(n) bf16 1024 | Y(n-2) bf16 512].
            gy = cpool.tile([128, NSLOT * 1536], bf16)
            g3 = gy[:, :].rearrange("a (k c) -> a k c", k=NSLOT, c=1536)

            def emit_mm2(n, yp):
                # MM2 for data tile n-MM2_LAG.
                slot2 = (n - MM2_LAG) % NSLOT
                g5 = g3[:, slot2, 0:1024].rearrange(
                    "a (r c s o) -> a s r c o", r=2, c=4, s=2, o=64)
                for tau in range(2):
                    for s in range(2):
                        nc.tensor.matmul(
                            yp[tau * 64:(tau + 1) * 64, :],
                            lhsT=b2[tau * 64:(tau + 1) * 64,
                                    s * 64:(s + 1) * 64],
                            rhs=g5[tau * 64:(tau + 1) * 64, s],
                            start=(s == 0), stop=(s == 1),
                            tile_position=(tau * 64, tau * 64),
                        )

            def emit_out_dma(m):
                # Ships Y of tiles [8m-2 .. 8m+5] (ring slots 8m..8m+7, lagged
                # by MM2_LAG) except the first group, which has no Y for the
                # two warmup slots.
                s0 = (GRP * m) % NSLOT
                if m == 0:
                    k0, k1 = MM2_LAG, GRP
                    ybase = 0
                else:
                    k0, k1 = s0, s0 + GRP
                    ybase = GRP * m - MM2_LAG
                nc.gpsimd.dma_start(
                    out=ydev[:, ybase * 512:(ybase + (k1 - k0)) * 512],
                    in_=g3[:, k0:k1, 1024:1536])

            # Variable input group sizes: small groups at the head so compute
            # starts after a ~128 KB DMA instead of 1 MB.
            head = [1, 1, 2, 4]
            tail = [4, 2, 1, 1]
            mid = (nt - sum(head) - sum(tail)) // GRP
            sizes = head + [GRP] * mid + tail
            assert sum(sizes) == nt, (sizes, nt)

            base = 0
            for glen in sizes:
                xs = xpool.tile([128, GRP * 512], bf16, tag="xs")
                nc.sync.dma_start(
                    out=xs[:, 0:glen * 512],
                    in_=xdev[:, base * 512:(base + glen) * 512])

                for t in range(glen):
                    n = base + t
                    ut = upool.tile([128, 1024], f32, tag="u")

                    # MM1: 16 matmuls, uniform 64x64 PE tiling. Quadrant
                    # (rho, tau) holds token 16g+4c+2rho+tau's X stationary;
                    # rho picks the PSUM bank, tau the output partitions.
                    for c in range(4):
                        for rho in range(2):
                            for tau in range(2):
                                nc.tensor.matmul(
                                    ut[tau * 64:(tau + 1) * 64,
                                       rho * 512 + c * 128:
                                       rho * 512 + (c + 1) * 128],
                                    lhsT=xs[rho * 64:(rho + 1) * 64,
                                            t * 512 + c * 128 + tau * 64:
                                            t * 512 + c * 128 + (tau + 1) * 64],
                                    rhs=a2[rho * 64:(rho + 1) * 64, :],
                                    start=True, stop=True,
                                    tile_position=(rho * 64, tau * 64),
                                )

                    if n >= MM2_LAG:
                        yt = ypool.tile([128, 512], f32, tag="y")
                        emit_mm2(n, yt)

                    # U evacuation: one 1024-col op into this tile's gy slot.
                    udst = g3[:, n % NSLOT, 0:1024]
                    if n % 6 == 5:
                        nc.vector.tensor_copy(udst, ut[:, :])
                    else:
                        nc.scalar.copy(udst, ut[:, :])
                    # Y evacuation (data of tile n-2) into this slot's Y half.
                    if n >= MM2_LAG:
                        nc.vector.tensor_copy(g3[:, n % NSLOT, 1024:1536],
                                              yt[:, :])

                    if n % GRP == GRP - 1:
                        emit_out_dma(n // GRP)
                base += glen

            # Epilogue: MM2 + Y drain for the last MM2_LAG tiles.
            for k in range(MM2_LAG):
                e = nt + k
                yt = ypool.tile([128, 512], f32, tag="y")
                emit_mm2(e, yt)
                ydst = g3[:, e % NSLOT, 1024:1536]
                if k % 2 == 0:
                    nc.scalar.copy(ydst, yt[:, :])
                else:
                    nc.vector.tensor_copy(ydst, yt[:, :])
            nc.gpsimd.dma_start(
                out=ydev[:, (nt - MM2_LAG) * 512:nt * 512],
                in_=g3[:, 0:MM2_LAG, 1024:1536])

    nc.finalize()
    return nc


def _get_nc(nt=NT):
    key = ("nc", nt)
    if key not in _cached:
        _cached[key] = _build_bass(nt)
    return _cached[key]


def _host_prep_x(xc):
    # xc: (TPC, 4096) f32 ->
    # xdev[rho*64+i, g*512 + c*128 + tau*64 + j] = xc[16g + 4c + 2rho + tau, i*64+j]
    x6 = xc.astype(BF16).reshape(NT, 4, 2, 2, IN1, IN2)   # g, c, rho, tau, i, j
    xd = x6.transpose(2, 4, 0, 1, 3, 5)                   # rho, i, g, c, tau, j
    return np.ascontiguousarray(xd).reshape(128, NT * 512)


def _host_post_y(yd, bias):
    # yd: (128, NT*512) bf16;
    # ydev[tau*64+q, g*512 + r*256 + c*64 + o] = y_mm[16g + 4c + 2r + tau, o*64+q]
    # bias is added here in f32 as part of the unpack epilogue.
    y6 = yd.reshape(2, OUT2, NT, 2, 4, OUT1)              # tau, q, g, r, c, o
    yc = y6.transpose(2, 4, 3, 0, 5, 1)                   # g, c, r, tau, o, q
    out = np.ascontiguousarray(yc).reshape(TPC, OUT1 * OUT2).astype(np.float32)
    out += bias
    return out


def _make_in_maps(x, A, B, bias):
    A = np.asarray(A, np.float32)
    B = np.asarray(B, np.float32)
    bias = np.asarray(bias, np.float32)
    xf = np.ascontiguousarray(x, np.float32).reshape(TOK, IN1 * IN2)

    at = A.transpose(2, 0, 1).reshape(IN1, NUM_SUM * OUT1)     # i, (s,o)
    a2d = np.ascontiguousarray(np.concatenate([at, at], 0)).astype(BF16)
    bt = B.transpose(2, 0, 1).reshape(IN2, NUM_SUM * OUT2)     # j, (s,q)
    b2d = np.ascontiguousarray(np.concatenate([bt, bt], 0)).astype(BF16)

    in_maps = []
    for cid in range(NCORES):
        xc = xf[cid * TPC:(cid + 1) * TPC]
        in_maps.append({
            "xdev": _host_prep_x(xc),
            "a2d": a2d,
            "b2d": b2d,
        })
    return in_maps


def _run(inputs, trace=False, **kw):
    from concourse.bass_utils import run_bass_kernel_spmd

    nc = _get_nc()
    in_maps = _make_in_maps(**inputs)
    res = run_bass_kernel_spmd(nc, in_maps, core_ids=list(range(NCORES)),
                               trace=trace, **kw)
    bias_f32 = np.asarray(inputs["bias"], np.float32)
    shards = [_host_post_y(np.asarray(res.results[c]["ydev"]), bias_f32)
              for c in range(NCORES)]
    y = np.concatenate(shards, 0).reshape(BATCH, SEQ, OUT1 * OUT2)
    return y, res


def kernel(x, A, B, bias):
    y, _ = _run(dict(x=x, A=A, B=B, bias=bias), trace=False)
    return y
